# revision 1
# baseline (speedup 1.0000x reference)
"""Trainium2 Bass kernel for a dense (length-1 sequence) Mamba block.

The reference computation reduces algebraically to:
    z   = x @ in_w                                  # (B, d_inner)
    g   = silu(z * c + b_eff)                       # per-channel scale/bias
    out = g @ out_w + out_b                         # (B, d_model)
with
    c     = conv_w[:, -1] + softplus(dt) * sum(B*C, -1) + Dp
    b_eff = (in_b * c) + conv_b
(c, b_eff are tiny per-channel vectors, computed on host.)

Strategy: data-parallel over 8 NeuronCores (batch 32768 -> 8 x 4096).
Per core, batch is processed in tiles of BT rows:
  phase T : PE-transpose x tiles into xT [d_model, BT] layout
  phase M1: z^T[di, b] accumulated over d_model via float32r (FP22) matmuls
            with in_w tiles as the stationary operand; Silu fused on ScalarE
            with per-partition scale/bias -> g [di, b]
  phase M2: out[b, dm] accumulated over d_inner with g slices as the
            stationary operand and out_w tiles moving (natural output
            layout; no output transpose needed); out_b added on drain.
"""

import numpy as np

import concourse.bass as bass
import concourse.tile as tile
from concourse import bacc, mybir
from concourse.bass_utils import run_bass_kernel_spmd

P = 128
B_FULL = 32768
DM = 2048
DI = 4096
N_CORES = 8
BS = B_FULL // N_CORES  # rows per core

F32 = mybir.dt.float32
F32R = mybir.dt.float32r
BF16 = mybir.dt.bfloat16
SILU = mybir.ActivationFunctionType.Silu


# float32r (FP22) tensors: same fp32 bytes in DRAM/numpy, but instructions
# producing them round to FP22 so the full-speed reduced-precision matmul
# path can consume them (walrus verifier requirement).


def build_nc(cfg):
    """Build the per-core Bass module. cfg: dict(BT=..., g_bf16=..., ow_bf16=...)"""
    BT = cfg["BT"]
    g_dt = BF16 if cfg["g_bf16"] else F32R
    ow_dt = BF16 if cfg["ow_bf16"] else F32R

    NBT = BS // BT          # batch tiles per core
    NB_SUB = BT // P        # 128-row subtiles per batch tile
    KT = DM // P            # k-tiles for matmul 1
    NDI = DI // P           # d_inner chunks of 128
    NDM = DM // 512         # d_model chunks of 512
    H = BT // 512           # moving-dim halves for matmul 1
    GRP = 4                 # psum banks used by M2 accumulation
    NGRP = NB_SUB // GRP

    nc = bacc.Bacc("TRN2", target_bir_lowering=False, debug=False,
                   num_devices=N_CORES)

    x_d = nc.dram_tensor("x", [BS, DM], F32R, kind="ExternalInput").ap()
    iw_d = nc.dram_tensor("iw", [DM, DI], F32R, kind="ExternalInput").ap()
    ow_d = nc.dram_tensor("ow", [DI, DM], ow_dt, kind="ExternalInput").ap()
    c_d = nc.dram_tensor("cpb", [P, NDI], F32, kind="ExternalInput").ap()
    b_d = nc.dram_tensor("bpb", [P, NDI], F32, kind="ExternalInput").ap()
    ob_d = nc.dram_tensor("ob", [P, DM], F32, kind="ExternalInput").ap()
    id_d = nc.dram_tensor("ident", [P, P], F32R, kind="ExternalInput").ap()
    out_d = nc.dram_tensor("out", [BS, DM], F32, kind="ExternalOutput").ap()

    DIG = 4                 # d_inner chunks per out_w DMA batch
    with tile.TileContext(nc) as tc:
        with (
            tc.tile_pool(name="const", bufs=1) as const,
            tc.tile_pool(name="xnat", bufs=2) as xnat,
            tc.tile_pool(name="xT", bufs=1) as xTp,
            tc.tile_pool(name="g", bufs=1) as gp,
            tc.tile_pool(name="iw", bufs=3) as iwp,
            tc.tile_pool(name="ow", bufs=3) as owp,
            tc.tile_pool(name="osb", bufs=2) as osbp,
            tc.tile_pool(name="psZ", bufs=3, space="PSUM") as psZ,
            tc.tile_pool(name="psO", bufs=5, space="PSUM") as psO,
        ):
            ident = const.tile([P, P], F32R)
            nc.sync.dma_start(ident[:], id_d)
            c_sb = const.tile([P, NDI], F32)
            nc.sync.dma_start(c_sb[:], c_d)
            b_sb = const.tile([P, NDI], F32)
            nc.sync.dma_start(b_sb[:], b_d)
            ob_sb = const.tile([P, DM], F32)
            nc.sync.dma_start(ob_sb[:], ob_d)

            xT = xTp.tile([P, KT, BT], F32R)
            g = gp.tile([P, NDI, BT], g_dt)

            def emit_T(t, bs):
                """Transpose one 128-row block of x[t] into xT."""
                xn = xnat.tile([P, DM], F32R, tag="xn", name="xn")
                nc.gpsimd.dma_start(xn[:], x_d[t * BT + bs * P:
                                               t * BT + (bs + 1) * P, :])
                for kq in range(KT // 4):
                    pst = psO.tile([P, 4, P], F32R, tag="ps_o", name="pst")
                    for q in range(4):
                        kt = kq * 4 + q
                        nc.tensor.transpose(
                            pst[:, q, :], xn[:, kt * P:(kt + 1) * P],
                            ident[:])
                    nc.vector.tensor_copy(
                        out=xT[:, kq * 4:(kq + 1) * 4, bs * P:(bs + 1) * P],
                        in_=pst[:])

            for t in range(NBT):
                if t == 0:
                    # prologue: transpose the first batch tile up front
                    for bs in range(NB_SUB):
                        emit_T(0, bs)

                # ---- phase M1: z^T = in_w^T @ x^T ; g = silu(z*c + b) ----
                for di in range(NDI):
                    iw_t = iwp.tile([P, KT, P], F32R)
                    nc.scalar.dma_start(
                        iw_t[:],
                        iw_d[:, di * P:(di + 1) * P].rearrange(
                            "(kt p) m -> p kt m", p=P))
                    zps = [psZ.tile([P, 512], F32, tag="zp", name=f"zp_{h}")
                           for h in range(H)]
                    for kt in range(KT):
                        for h in range(H):
                            nc.tensor.matmul(
                                zps[h][:],
                                iw_t[:, kt, :],
                                xT[:, kt, h * 512:(h + 1) * 512],
                                start=(kt == 0), stop=(kt == KT - 1))
                    for h in range(H):
                        nc.scalar.activation(
                            g[:, di, h * 512:(h + 1) * 512], zps[h][:], SILU,
                            bias=b_sb[:, di:di + 1], scale=c_sb[:, di:di + 1])

                # ---- phase M2: out = g^T @ out_w + out_b ----
                # T-units for batch tile t+1 interleaved after each psum
                # group: transposes fill PE slack, x DMAs spread over the
                # whole M2 window.
                ui = 0
                for dmc in range(NDM):
                    for grp in range(NGRP):
                        ops = [psO.tile([P, 512], F32, tag="ps_o",
                                        name=f"ops_{j}")
                               for j in range(GRP)]
                        for dg in range(NDI // DIG):
                            ow_t = owp.tile([P, DIG, 512], ow_dt)
                            nc.sync.dma_start(
                                ow_t[:],
                                ow_d[dg * DIG * P:(dg + 1) * DIG * P,
                                     dmc * 512:(dmc + 1) * 512].rearrange(
                                         "(s p) n -> p s n", p=P))
                            for s in range(DIG):
                                di = dg * DIG + s
                                for j in range(GRP):
                                    bs = grp * GRP + j
                                    nc.tensor.matmul(
                                        ops[j][:],
                                        g[:, di, bs * P:(bs + 1) * P],
                                        ow_t[:, s, :],
                                        start=(di == 0),
                                        stop=(di == NDI - 1))
                        osb = osbp.tile([P, GRP, 512], F32)
                        for j in range(GRP):
                            nc.vector.tensor_tensor(
                                osb[:, j, :], ops[j][:],
                                ob_sb[:, dmc * 512:(dmc + 1) * 512],
                                mybir.AluOpType.add)
                        r0 = t * BT + grp * GRP * P
                        nc.scalar.dma_start(
                            out_d[r0:r0 + GRP * P,
                                  dmc * 512:(dmc + 1) * 512].rearrange(
                                      "(s p) n -> p s n", p=P),
                            osb[:])
                        if t + 1 < NBT and ui < NB_SUB:
                            emit_T(t + 1, ui)
                            ui += 1
    nc.compile()
    return nc


_NC_CACHE = {}


def _get_nc(key):
    if key not in _NC_CACHE:
        cfg = dict(BT=key[0], g_bf16=key[1], ow_bf16=key[2])
        _NC_CACHE[key] = build_nc(cfg)
    return _NC_CACHE[key]


# default config: fp32r matmul-1, bf16 g + out_w for matmul-2, BT=1024
CONFIG = (1024, True, True)


def _softplus(v):
    return np.logaddexp(0.0, v)


def kernel(x, in_w, in_b, conv_w, conv_b, A_log, B, C, Dp, dt, out_w, out_b):
    x = np.asarray(x, dtype=np.float32)
    in_w = np.ascontiguousarray(np.asarray(in_w, dtype=np.float32))
    out_w = np.asarray(out_w, dtype=np.float32)

    # host precompute of the per-channel SSM/conv collapse
    c = (np.asarray(conv_w, np.float32)[:, -1]
         + _softplus(np.asarray(dt, np.float32))
         * np.sum(np.asarray(B, np.float32) * np.asarray(C, np.float32), -1)
         + np.asarray(Dp, np.float32))
    b_eff = np.asarray(in_b, np.float32) * c + np.asarray(conv_b, np.float32)

    # [128, DI//128] partition-major layouts for per-partition scale/bias
    c_pb = np.ascontiguousarray(c.reshape(DI // P, P).T)
    b_pb = np.ascontiguousarray(b_eff.reshape(DI // P, P).T)
    ob_rep = np.ascontiguousarray(
        np.broadcast_to(np.asarray(out_b, np.float32), (P, DM)))

    key = CONFIG
    nc = _get_nc(key)
    if key[2]:
        import ml_dtypes
        ow_arr = out_w.astype(ml_dtypes.bfloat16)
    else:
        ow_arr = np.ascontiguousarray(out_w)

    in_maps = []
    for i in range(N_CORES):
        in_maps.append({
            "x": np.ascontiguousarray(x[i * BS:(i + 1) * BS]),
            "iw": in_w,
            "ow": ow_arr,
            "cpb": c_pb,
            "bpb": b_pb,
            "ob": ob_rep,
            "ident": np.eye(P, dtype=np.float32),
        })
    out = np.empty((B_FULL, DM), dtype=np.float32)
    try:
        res = run_bass_kernel_spmd(nc, in_maps, core_ids=list(range(N_CORES)))
        for i in range(N_CORES):
            out[i * BS:(i + 1) * BS] = res.results[i]["out"]
    except Exception:
        # The accelerator occasionally hits a transient unrecoverable fault
        # that poisons this process's PJRT client; a fresh process recovers.
        # Retry the device execution in a subprocess.
        _run_in_subprocess(in_maps, out)
    return out


def _run_in_subprocess(in_maps, out):
    import pickle
    import subprocess
    import sys
    import tempfile

    with tempfile.TemporaryDirectory() as td:
        in_path = f"{td}/in.pkl"
        out_path = f"{td}/out.npy"
        with open(in_path, "wb") as f:
            pickle.dump({"config": CONFIG, "in_maps": in_maps}, f,
                        protocol=pickle.HIGHEST_PROTOCOL)
        for attempt in range(3):
            r = subprocess.run(
                [sys.executable, __file__, "--worker", in_path, out_path],
                capture_output=True)
            if r.returncode == 0:
                break
            if attempt == 2:
                raise RuntimeError(
                    f"device worker failed 3x: {r.stderr[-2000:]!r}")
        out[:] = np.load(out_path)


def _worker_main(in_path, out_path):
    import pickle
    with open(in_path, "rb") as f:
        job = pickle.load(f)
    nc = _get_nc(tuple(job["config"]))
    res = run_bass_kernel_spmd(nc, job["in_maps"],
                               core_ids=list(range(N_CORES)))
    out = np.empty((B_FULL, DM), dtype=np.float32)
    for i in range(N_CORES):
        out[i * BS:(i + 1) * BS] = res.results[i]["out"]
    np.save(out_path, out)


if __name__ == "__main__":
    import sys as _sys
    if len(_sys.argv) == 4 and _sys.argv[1] == "--worker":
        _worker_main(_sys.argv[2], _sys.argv[3])



# revision 2
# speedup vs baseline: 1.1102x; 1.1102x over previous
"""Trainium2 Bass kernel for a dense (length-1 sequence) Mamba block.

The reference computation reduces algebraically to:
    z   = x @ in_w                                  # (B, d_inner)
    g   = silu(z * c + b_eff)                       # per-channel scale/bias
    out = g @ out_w + out_b                         # (B, d_model)
with
    c     = conv_w[:, -1] + softplus(dt) * sum(B*C, -1) + Dp
    b_eff = (in_b * c) + conv_b
(c, b_eff are tiny per-channel vectors, computed on host.)

Strategy: data-parallel over 8 NeuronCores (batch 32768 -> 8 x 4096).
All-bf16 datapath (inputs converted on host; well within tolerance).
Per core, batch is processed in tiles of BT rows:
  x^T tiles are produced by the XBAR DMA-transpose (HBM -> SBUF) so the
  PE array spends zero cycles on transposes.
  phase M1: z^T[di, b] accumulated over d_model; Silu fused on ScalarE
            with per-partition scale/bias -> g [di, b] (bf16)
  phase M2: out[b, dm] accumulated over d_inner with g slices as the
            stationary operand and out_w tiles moving (natural output
            layout); out_b added on drain.
The kernel is PE-bound (2 x 4096x2048x4096 MACs/core); everything else
overlaps under the matmul stream.
"""

import numpy as np

import concourse.bass as bass
import concourse.tile as tile
from concourse import bacc, mybir
from concourse.bass_utils import run_bass_kernel_spmd

P = 128
B_FULL = 32768
DM = 2048
DI = 4096
N_CORES = 8
BS = B_FULL // N_CORES  # rows per core

F32 = mybir.dt.float32
BF16 = mybir.dt.bfloat16
SILU = mybir.ActivationFunctionType.Silu

KT = DM // P            # 16 k-chunks for matmul 1
NDI = DI // P           # 32 d_inner chunks of 128
NDM = DM // 512         # 4 d_model chunks of 512
DIG = 4                 # d_inner chunks per out_w DMA batch
NDG = NDI // DIG        # 8 out_w loads per (dmc, grp)
GRP = 4                 # psum banks used by M2 accumulation


def build_nc(cfg):
    """Build the per-core Bass module. cfg: dict(BT=...)"""
    BT = cfg["BT"]
    NBT = BS // BT          # batch tiles per core
    NB_SUB = BT // P        # 128-row subtiles per batch tile
    H = BT // 512           # moving-dim halves for matmul 1
    NGRP = NB_SUB // GRP

    nc = bacc.Bacc("TRN2", target_bir_lowering=False, debug=False,
                   num_devices=N_CORES)

    x_d = nc.dram_tensor("x16", [BS, DM], BF16, kind="ExternalInput").ap()
    iw_d = nc.dram_tensor("iwp", [NDI, P, KT * P], BF16,
                          kind="ExternalInput").ap()
    ow_d = nc.dram_tensor("owp", [NDM, NDG, P, DIG * 512], BF16,
                          kind="ExternalInput").ap()
    c_d = nc.dram_tensor("cpb", [P, NDI], F32, kind="ExternalInput").ap()
    b_d = nc.dram_tensor("bpb", [P, NDI], F32, kind="ExternalInput").ap()
    ob_d = nc.dram_tensor("ob", [P, DM], F32, kind="ExternalInput").ap()
    out_d = nc.dram_tensor("out", [BS, DM], F32, kind="ExternalOutput").ap()

    with tile.TileContext(nc) as tc:
        with (
            tc.tile_pool(name="const", bufs=1) as const,
            tc.tile_pool(name="xT", bufs=1) as xTp,
            tc.tile_pool(name="g", bufs=1) as gp,
            tc.tile_pool(name="iw", bufs=4) as iwp,
            tc.tile_pool(name="ow", bufs=4) as owp,
            tc.tile_pool(name="osb", bufs=2) as osbp,
            tc.tile_pool(name="psZ", bufs=3, space="PSUM") as psZ,
            tc.tile_pool(name="psO", bufs=5, space="PSUM") as psO,
        ):
            c_sb = const.tile([P, NDI], F32)
            nc.sync.dma_start(c_sb[:], c_d)
            b_sb = const.tile([P, NDI], F32)
            nc.sync.dma_start(b_sb[:], b_d)
            ob_sb = const.tile([P, DM], F32)
            nc.sync.dma_start(ob_sb[:], ob_d)

            xT = xTp.tile([P, KT, BT], BF16)
            g = gp.tile([P, NDI, BT], BF16)

            def emit_xT(t, kt):
                """XBAR DMA-transpose one [BT, 128] slab of x into xT."""
                nc.scalar.dma_start(
                    xT[:, kt, :],
                    x_d[t * BT:(t + 1) * BT, kt * P:(kt + 1) * P],
                    transpose=True)

            for t in range(NBT):
                if t == 0:
                    for kt in range(KT):
                        emit_xT(0, kt)

                # ---- phase M1: z^T = in_w^T @ x^T ; g = silu(z*c + b) ----
                for di in range(NDI):
                    iw_t = iwp.tile([P, KT, P], BF16)
                    nc.sync.dma_start(iw_t[:], iw_d[di].rearrange(
                        "p (kt m) -> p kt m", kt=KT))
                    zps = [psZ.tile([P, 512], F32, tag="zp", name=f"zp_{h}")
                           for h in range(H)]
                    for kt in range(KT):
                        for h in range(H):
                            nc.tensor.matmul(
                                zps[h][:],
                                iw_t[:, kt, :],
                                xT[:, kt, h * 512:(h + 1) * 512],
                                start=(kt == 0), stop=(kt == KT - 1))
                    for h in range(H):
                        nc.scalar.activation(
                            g[:, di, h * 512:(h + 1) * 512], zps[h][:], SILU,
                            bias=b_sb[:, di:di + 1], scale=c_sb[:, di:di + 1])

                # ---- phase M2: out = g^T @ out_w + out_b ----
                # x^T DMA-transposes for batch tile t+1 are spread over the
                # M2 window (2 per psum group).
                ui = 0
                for dmc in range(NDM):
                    for grp in range(NGRP):
                        ops = [psO.tile([P, 512], F32, tag="ps_o",
                                        name=f"ops_{j}")
                               for j in range(GRP)]
                        for dg in range(NDG):
                            ow_t = owp.tile([P, DIG, 512], BF16)
                            nc.sync.dma_start(
                                ow_t[:], ow_d[dmc, dg].rearrange(
                                    "p (s n) -> p s n", s=DIG))
                            for s in range(DIG):
                                di = dg * DIG + s
                                for j in range(GRP):
                                    bs = grp * GRP + j
                                    nc.tensor.matmul(
                                        ops[j][:],
                                        g[:, di, bs * P:(bs + 1) * P],
                                        ow_t[:, s, :],
                                        start=(di == 0),
                                        stop=(di == NDI - 1))
                        osb = osbp.tile([P, GRP, 512], F32)
                        for j in range(GRP):
                            nc.vector.tensor_tensor(
                                osb[:, j, :], ops[j][:],
                                ob_sb[:, dmc * 512:(dmc + 1) * 512],
                                mybir.AluOpType.add)
                        r0 = t * BT + grp * GRP * P
                        nc.scalar.dma_start(
                            out_d[r0:r0 + GRP * P,
                                  dmc * 512:(dmc + 1) * 512].rearrange(
                                      "(s p) n -> p s n", p=P),
                            osb[:])
                        if t + 1 < NBT:
                            for _ in range(2):
                                if ui < KT:
                                    emit_xT(t + 1, ui)
                                    ui += 1
    nc.compile()
    return nc


_NC_CACHE = {}


def _get_nc(key):
    if key not in _NC_CACHE:
        cfg = dict(BT=key[0])
        _NC_CACHE[key] = build_nc(cfg)
    return _NC_CACHE[key]


CONFIG = (1024,)


def _softplus(v):
    return np.logaddexp(0.0, v)


def prep_inputs(x, in_w, in_b, conv_w, conv_b, A_log, B, C, Dp, dt,
                out_w, out_b):
    """Host-side prep shared by kernel() and the test harness."""
    import ml_dtypes
    bf16 = ml_dtypes.bfloat16

    x16 = np.asarray(x, np.float32).astype(bf16)

    # in_w [DM, DI] -> iwp [NDI, P(dm-in-chunk), KT*P(di-in-chunk)]
    iw = np.asarray(in_w, np.float32).astype(bf16)
    iwp = np.ascontiguousarray(
        iw.reshape(KT, P, NDI, P).transpose(2, 1, 0, 3).reshape(
            NDI, P, KT * P))

    # out_w [DI, DM] -> owp [NDM, NDG, P(di-in-chunk), DIG*512]
    ow = np.asarray(out_w, np.float32).astype(bf16)
    owp = np.ascontiguousarray(
        ow.reshape(NDG, DIG, P, NDM, 512).transpose(3, 0, 2, 1, 4).reshape(
            NDM, NDG, P, DIG * 512))

    # host precompute of the per-channel SSM/conv collapse
    c = (np.asarray(conv_w, np.float32)[:, -1]
         + _softplus(np.asarray(dt, np.float32))
         * np.sum(np.asarray(B, np.float32) * np.asarray(C, np.float32), -1)
         + np.asarray(Dp, np.float32))
    b_eff = np.asarray(in_b, np.float32) * c + np.asarray(conv_b, np.float32)

    c_pb = np.ascontiguousarray(c.reshape(NDI, P).T)
    b_pb = np.ascontiguousarray(b_eff.reshape(NDI, P).T)
    ob_rep = np.ascontiguousarray(
        np.broadcast_to(np.asarray(out_b, np.float32), (P, DM)))

    in_maps = []
    for i in range(N_CORES):
        in_maps.append({
            "x16": np.ascontiguousarray(x16[i * BS:(i + 1) * BS]),
            "iwp": iwp,
            "owp": owp,
            "cpb": c_pb,
            "bpb": b_pb,
            "ob": ob_rep,
        })
    return in_maps


def kernel(x, in_w, in_b, conv_w, conv_b, A_log, B, C, Dp, dt, out_w, out_b):
    in_maps = prep_inputs(x, in_w, in_b, conv_w, conv_b, A_log, B, C, Dp,
                          dt, out_w, out_b)
    nc = _get_nc(CONFIG)
    out = np.empty((B_FULL, DM), dtype=np.float32)
    try:
        res = run_bass_kernel_spmd(nc, in_maps, core_ids=list(range(N_CORES)))
        for i in range(N_CORES):
            out[i * BS:(i + 1) * BS] = res.results[i]["out"]
    except Exception:
        # The accelerator occasionally hits a transient unrecoverable fault
        # that poisons this process's PJRT client; a fresh process recovers.
        # Retry the device execution in a subprocess.
        _run_in_subprocess(in_maps, out)
    return out


def _run_in_subprocess(in_maps, out):
    import pickle
    import subprocess
    import sys
    import tempfile

    with tempfile.TemporaryDirectory() as td:
        in_path = f"{td}/in.pkl"
        out_path = f"{td}/out.npy"
        with open(in_path, "wb") as f:
            pickle.dump({"config": CONFIG, "in_maps": in_maps}, f,
                        protocol=pickle.HIGHEST_PROTOCOL)
        for attempt in range(3):
            r = subprocess.run(
                [sys.executable, __file__, "--worker", in_path, out_path],
                capture_output=True)
            if r.returncode == 0:
                break
            if attempt == 2:
                raise RuntimeError(
                    f"device worker failed 3x: {r.stderr[-2000:]!r}")
        out[:] = np.load(out_path)


def _worker_main(in_path, out_path):
    import pickle
    with open(in_path, "rb") as f:
        job = pickle.load(f)
    nc = _get_nc(tuple(job["config"]))
    res = run_bass_kernel_spmd(nc, job["in_maps"],
                               core_ids=list(range(N_CORES)))
    out = np.empty((B_FULL, DM), dtype=np.float32)
    for i in range(N_CORES):
        out[i * BS:(i + 1) * BS] = res.results[i]["out"]
    np.save(out_path, out)


if __name__ == "__main__":
    import sys as _sys
    if len(_sys.argv) == 4 and _sys.argv[1] == "--worker":
        _worker_main(_sys.argv[2], _sys.argv[3])
